# revision 19
# baseline (speedup 1.0000x reference)
"""Trainium2 Bass kernel for nn_BlockSparseMLP (MoE gated MLP, E=8, top-2).

Strategy: expert parallelism over 8 NeuronCores. The router matmul
(x @ w_router, 67 MFLOP out of the 206 GFLOP dense total) plus the
top-2 dispatch/gather and the final scatter-add combine run on the
host; each core runs the full gated MLP (gate/up, silu*up, down,
weighted by the routing prob) for the 512 tokens routed to its expert
(capacity-factor dispatch; overflow beyond 512 tokens/expert takes an
exact fp32 host path). Matmul operands are bf16 (1 col/cycle on the
PE, same rate as float32r, but fast-weight-load keeps LDWEIGHTS off
the critical path and HBM traffic halves); PSUM accumulation and the
silu/mul/scale epilogue run fp32, outputs are stored bf16.

Per-core device program (C = 512 tokens, token-major tiles of 128):
  phase 1 (gate/up): stream w_gate/w_up in [128, gate 256|up 256]
    chunks, two k-tiles per 256KB DMA; weights stationary, xT moving
    (N=512). silu(gate)*up fused on ACT+DVE into aT ([I, C] bf16,
    SBUF-resident, one tile per chunk so phase 2's early reads don't
    depend on the last chunk's writes).
  phase 2 (down): stream w_down in two-k-tile chunks (moving, N=512
    per matmul), aT tiles stationary, accumulate over I into
    [tokens, 512] psum tiles, scale by the per-token routing weight,
    DMA out. The last H-chunk's w_down is fully SBUF-resident (loaded
    mid-phase-2) so that chunk runs token-major and its outputs drain
    one at a time instead of all four serializing after the final
    matmul.

Scheduling notes (each worth microseconds on a 352us kernel):
  - Sync issues ALL streamed weight DMAs and nothing else: weight
    dma_starts block on tile-pool slots once the prefetch races ahead
    of the PE, and anything queued behind them on that engine stalls.
    An earlier revision alternated weight loads sync/scalar; the silu
    for chunk N sat behind chunk N+2's slot-blocked load, psum filled,
    and the PE idled 6us and re-throttled its HAM clock.
  - Scalar: x loads (16 x 128KB at the head; the first matmul only
    waits on one k-tile), then the nh=0 w_down head into a dedicated
    pool (issued after chunk 1's silu, clear of the cold-start HBM
    ramp), then the silus, then most output stores.
  - One PSUM pool spans both phases and the down-projection tiles
    reuse the gate/up tags: FIFO slot recycling then hands phase 2
    banks that were freed a chunk earlier, so the first down matmul
    does not wait for the last chunk's psum drain.
  - GpSimd only loads the tiny routing-weight tile.

Measured on trn2: 352us HW exec (PE stream floor for this shape is
1536 N=512 matmuls = 331.5us; remainder is NEFF preamble ~7us, cold
HAM ramp ~2us, startup DMA ~3us, output tail + drain ~5us, ramp
transients ~3us). Rel err vs the fp32 reference ~4.4e-3.
"""

import sys
import functools

sys.path.insert(0, "/opt/trn_rl_repo")

import numpy as np

T, H, II, E, TOPK = 2048, 2048, 4096, 8, 2
NCORES = 8
B0 = 512        # token capacity per expert (moving N for gate/up)
KT = H // 128   # 16 contraction tiles for gate/up
MTI = II // 128  # 32 I tiles
NMC = II // 256  # 16 weight chunks of [gate 256 | up 256]
KI = II // 128  # 32 contraction tiles for down
NH = H // 512   # 4 output column chunks
NT = B0 // 128  # 4 token tiles


@functools.lru_cache(maxsize=2)
def _build(nb1: int = 0):
    """Build the SPMD Bass program (capacity fixed at B0=512 tokens)."""
    assert nb1 == 0
    import concourse.mybir as mybir
    import concourse.tile as tile
    from concourse import bacc

    f32 = mybir.dt.float32
    bf16 = mybir.dt.bfloat16

    nc = bacc.Bacc(None)
    # x, token-major transposed, packed 4 k-tiles per DMA: [KT/4, 128, 4*B0]
    xT = nc.declare_dram_parameter("xT0", [KT // 4, 128, 4 * B0], bf16, isOutput=False)
    # gate|up chunks, 2 k-tiles per DMA: [NMC, KT/2, 128, 1024]
    wgu = nc.declare_dram_parameter("wgu", [NMC, KT // 2, 128, 1024], bf16, isOutput=False)
    # down chunks, 2 k-tiles per DMA: [NH, KI/2, 128, 1024]
    wd = nc.declare_dram_parameter("wd", [NH, KI // 2, 128, 1024], bf16, isOutput=False)
    rw = nc.declare_dram_parameter("rw", [128, NT], f32, isOutput=False)
    # output, contiguous per (nh, mt) tile: [NH, NT, 128, 512]
    dout = nc.declare_dram_parameter("d", [NH, NT, 128, 512], bf16, isOutput=True)

    SILU = mybir.ActivationFunctionType.Silu

    NPF = 8  # wd pairs of nh=0 prefetched at program start (via GpSimd)

    with tile.TileContext(nc) as tc:
        with (
            tc.tile_pool(name="pers", bufs=1) as pers,
            tc.tile_pool(name="wpool", bufs=14) as wpool,
            tc.tile_pool(name="wd0p", bufs=1) as wd0p,
            tc.tile_pool(name="ps", bufs=1, space="PSUM") as ps,
        ):
            aT0 = [pers.tile([128, 2, B0], bf16, name=f"aT0_{mc}")
                   for mc in range(NMC)]
            rwt = pers.tile([128, NT], f32)
            nc.gpsimd.dma_start(rwt[:], rw[:])
            # Warmup: a few matmuls on a zeroed tile, queued ahead of the
            # first real one. They run inside the otherwise-idle window
            # while the first weight DMA is in flight and start the PE's
            # HAM activity window ~3us early, so the 1.2->2.4GHz
            # un-throttle fires before (not 3.4us into) the real stream.
            warm = pers.tile([128, 512], bf16, name="warm")
            nc.gpsimd.memset(warm[:], 0)
            for i in range(5):
                pw = ps.tile([128, 512], f32, name="pwarm",
                             tag=("pg0" if i % 2 == 0 else "pu0"), bufs=4)
                nc.tensor.matmul(pw[:], warm[:, :128], warm[:],
                                 start=True, stop=True)
            # nh=0's w_down head gets a dedicated pool: the shared wpool's
            # slots only free near the end of phase 1 (they pace the wgu
            # prefetch), so streaming the first wd chunk through it would
            # stall the phase-1 -> phase-2 handoff. Loads are issued on
            # Scalar after the x loads: early enough to land mid-phase-1,
            # late enough not to steal HBM bandwidth from the startup ramp.
            wd0 = [wd0p.tile([128, 1024], bf16, name=f"wd0_{kk}")
                   for kk in range(NPF)]

            with (
                tc.tile_pool(name="xp", bufs=1) as xp,
                tc.tile_pool(name="sp", bufs=2) as sp,
            ):
                # x as 16 single k-tiles (not wider packs): the first matmul
                # only waits on a 128KB load, and the issues sit on Scalar,
                # which has nothing else to do until the first silu.
                xt = [xp.tile([128, B0], bf16, name=f"xt_{k}") for k in range(KT)]
                for k in range(KT):
                    nc.scalar.dma_start(xt[k][:], xT[k // 4][:, (k % 4) * B0:(k % 4 + 1) * B0])

                for mc in range(NMC):
                    # wb[k] -> (tile, column offset of k's gate|up block)
                    wb = []
                    for kk in range(KT // 2):
                        if mc < 2:
                            # cold-HBM ramp: single-k 128KB loads halve
                            # the latency to the tile the PE waits on
                            for half in range(2):
                                wbk = wpool.tile([128, 512], bf16,
                                                 name="wbs", tag="ws", bufs=8)
                                nc.sync.dma_start(
                                    wbk[:], wgu[mc, kk][:, half * 512:(half + 1) * 512])
                                wb.append((wbk, 0))
                        else:
                            wbk = wpool.tile([128, 1024], bf16, name="wbk", tag="w")
                            nc.sync.dma_start(wbk[:], wgu[mc, kk])
                            wb.append((wbk, 0))
                            wb.append((wbk, 512))

                    pg0 = [ps.tile([128, B0], f32, name="pg0", tag="pg0", bufs=4)
                           for _ in range(2)]
                    pu0 = [ps.tile([128, B0], f32, name="pu0", tag="pu0", bufs=4)
                           for _ in range(2)]

                    for k in range(KT):
                        st = dict(start=(k == 0), stop=(k == KT - 1))
                        wsl, wo = wb[k]
                        for j in range(2):
                            nc.tensor.matmul(
                                pg0[j][:], wsl[:, wo + j * 128:wo + (j + 1) * 128],
                                xt[k][:], **st
                            )
                        for j in range(2):
                            nc.tensor.matmul(
                                pu0[j][:],
                                wsl[:, wo + 256 + j * 128:wo + 256 + (j + 1) * 128],
                                xt[k][:], **st
                            )

                    for j in range(2):
                        sg = sp.tile([128, B0], f32, name="sg", tag="sg")
                        nc.scalar.activation(sg[:], pg0[j][:], SILU)
                        nc.vector.tensor_mul(aT0[mc][:, j, :], sg[:], pu0[j][:])
                    if mc == 1:
                        # nh=0's wd head loads go here on Scalar: after the
                        # startup ramp's bandwidth crunch, long before use
                        for kk in range(NPF):
                            nc.scalar.dma_start(wd0[kk][:], wd[0, kk])

            with (
                tc.tile_pool(name="op", bufs=6) as op,
                tc.tile_pool(name="wlast", bufs=1) as wlast,
            ):
                # The last nh chunk's w_down (4MB) is preloaded whole, so
                # the chunk can run token-major and its outputs drain one
                # at a time instead of all four serializing after the final
                # matmul. Issues go on Sync AFTER nh=0's loads (below), so
                # they execute ~80us before use without competing with the
                # startup ramp.
                wl = [wlast.tile([128, 1024], bf16, name=f"wl_{kk}")
                      for kk in range(KI // 2)]

                for nh in range(NH - 1):
                    pd = [ps.tile([128, 512], f32, name="pd",
                                  tag=("pg0" if mt % 2 == 0 else "pu0"), bufs=4)
                          for mt in range(NT)]
                    for kk in range(KI // 2):
                        if nh == 0 and kk < NPF:
                            wdk = wd0[kk]
                        else:
                            wdk = wpool.tile([128, 1024], bf16, name="wdk", tag="w")
                            nc.sync.dma_start(wdk[:], wd[nh, kk])
                        for half in range(2):
                            k = 2 * kk + half
                            st = dict(start=(k == 0), stop=(k == KI - 1))
                            mv = wdk[:, half * 512:(half + 1) * 512]
                            for mt in range(NT):
                                nc.tensor.matmul(
                                    pd[mt][:],
                                    aT0[k // 2][:, k % 2, mt * 128:(mt + 1) * 128],
                                    mv, **st
                                )
                    if nh == 0:
                        for kk in range(KI // 2):
                            nc.sync.dma_start(wl[kk][:], wd[NH - 1, kk])
                    for mt in range(NT):
                        ot = op.tile([128, 512], bf16, name="ot", tag="ot")
                        nc.vector.tensor_scalar_mul(ot[:], pd[mt][:], rwt[:, mt:mt + 1])
                        nc.scalar.dma_start(dout[nh, mt], ot[:])

                # last nh chunk: token-major, alternating scale/store engines
                nh = NH - 1
                for mt in range(NT):
                    pdt = ps.tile([128, 512], f32, name="pd",
                                  tag=("pg0" if mt % 2 == 0 else "pu0"), bufs=4)
                    for k in range(KI):
                        st = dict(start=(k == 0), stop=(k == KI - 1))
                        nc.tensor.matmul(
                            pdt[:],
                            aT0[k // 2][:, k % 2, mt * 128:(mt + 1) * 128],
                            wl[k // 2][:, (k % 2) * 512:(k % 2 + 1) * 512], **st
                        )
                    ot = op.tile([128, 512], bf16, name="ot", tag="ot")
                    if mt % 2 == 0:
                        nc.vector.tensor_scalar_mul(ot[:], pdt[:], rwt[:, mt:mt + 1])
                        nc.scalar.dma_start(dout[nh, mt], ot[:])
                    else:
                        nc.scalar.activation(
                            ot[:], pdt[:], mybir.ActivationFunctionType.Copy,
                            scale=rwt[:, mt:mt + 1]
                        )
                        nc.sync.dma_start(dout[nh, mt], ot[:])

    nc.compile()
    return nc


@functools.lru_cache(maxsize=2)
def _get_exec(nb1: int = 0):
    """Compile the Bass program and return (nc, run_fn) with a cached jit.

    run_fn(in_maps) -> list of per-core {"d": np.ndarray}. Mirrors
    bass2jax.run_bass_via_pjrt's multi-core branch, but keeps the jitted
    function alive across kernel() calls so repeat invocations skip XLA
    and NEFF compilation.
    """
    import jax
    import concourse.mybir as mybir
    from concourse import bass2jax

    nc = _build(nb1)
    bass2jax.install_neuronx_cc_hook()

    partition_name = nc.partition_id_tensor.name if nc.partition_id_tensor else None
    in_names, out_names, out_avals = [], [], []
    zero_out_shapes = []
    for alloc in nc.m.functions[0].allocations:
        if not isinstance(alloc, mybir.MemoryLocationSet):
            continue
        name = alloc.memorylocations[0].name
        if alloc.kind == "ExternalInput":
            if name != partition_name:
                in_names.append(name)
        elif alloc.kind == "ExternalOutput":
            shape = tuple(alloc.tensor_shape)
            dtype = mybir.dt.np(alloc.dtype)
            out_names.append(name)
            out_avals.append(jax.core.ShapedArray(shape, dtype))
            zero_out_shapes.append((shape, dtype))
    n_params = len(in_names)
    n_outs = len(out_names)
    all_names = list(in_names) + list(out_names)
    if partition_name is not None:
        all_names.append(partition_name)
    donate = tuple(range(n_params, n_params + n_outs))

    def _body(*args):
        operands = list(args)
        if partition_name is not None:
            operands.append(bass2jax.partition_id_tensor())
        outs = bass2jax._bass_exec_p.bind(
            *operands,
            out_avals=tuple(out_avals),
            in_names=tuple(all_names),
            out_names=tuple(out_names),
            lowering_input_output_aliases=(),
            sim_require_finite=True,
            sim_require_nnan=True,
            nc=nc,
        )
        return tuple(outs)

    devices = jax.devices()[:NCORES]
    assert len(devices) == NCORES, f"need {NCORES} devices, have {len(jax.devices())}"
    mesh = bass2jax.Mesh(np.asarray(devices), ("core",))
    in_specs = (bass2jax.PartitionSpec("core"),) * (n_params + n_outs)
    out_specs = (bass2jax.PartitionSpec("core"),) * n_outs
    sharded = jax.jit(
        bass2jax.shard_map(
            _body, mesh=mesh, in_specs=in_specs, out_specs=out_specs, check_rep=False
        ),
        donate_argnums=donate,
        keep_unused=True,
    )

    def run_fn(in_maps):
        concat_in = [
            np.concatenate([np.asarray(m[name]) for m in in_maps], axis=0)
            for name in in_names
        ]
        zeros = [
            np.zeros((shape[0] * NCORES,) + shape[1:], dtype)
            for shape, dtype in zero_out_shapes
        ]
        out_arrs = sharded(*concat_in, *zeros)
        results = []
        for c in range(NCORES):
            res = {}
            for i, name in enumerate(out_names):
                arr = np.asarray(out_arrs[i])
                per = arr.shape[0] // NCORES
                res[name] = arr[c * per:(c + 1) * per]
            results.append(res)
        return results

    return nc, run_fn


def _route(x, w_router):
    """Top-2 routing: expert ids + softmax weights, matching jax.lax.top_k
    (descending, ties to the lower index) + jax.nn.softmax."""
    logits = x.astype(np.float64) @ w_router.astype(np.float64)
    top2 = np.argsort(-logits, axis=1, kind="stable")[:, :TOPK]
    vals = np.take_along_axis(logits, top2, 1).astype(np.float32)
    e = np.exp(vals - vals.max(axis=1, keepdims=True))
    w = (e / e.sum(axis=1, keepdims=True)).astype(np.float32)
    return top2, w


def _reference_numpy(x, w_router, w_gate, w_up, w_down):
    """Correct-but-slow dense fallback for shapes the device program doesn't cover."""
    x = x.astype(np.float32)
    logits = x @ w_router.astype(np.float32)
    n_exp = w_gate.shape[0]
    k = min(TOPK, n_exp)
    top = np.argsort(-logits, axis=1, kind="stable")[:, :k]
    vals = np.take_along_axis(logits, top, 1)
    ex = np.exp(vals - vals.max(1, keepdims=True))
    ww = (ex / ex.sum(1, keepdims=True)).astype(np.float32)
    w_dense = np.zeros_like(logits)
    t_ids = np.arange(x.shape[0])[:, None]
    w_dense[t_ids, top] = ww
    out = np.zeros((x.shape[0], w_down.shape[-1]), np.float32)
    for e in range(n_exp):
        g = x @ w_gate[e]
        u = x @ w_up[e]
        a = (g / (1.0 + np.exp(-g))) * u
        out += w_dense[:, e:e + 1] * (a @ w_down[e])
    return out


def _to_bf16(a):
    """f32 -> bf16 with round-to-nearest-even (vectorized integer trick)."""
    import ml_dtypes
    b = np.ascontiguousarray(a, np.float32).view(np.uint32)
    r = ((b >> 16) & 1) + np.uint32(0x7FFF)
    return ((b + r) >> 16).astype(np.uint16).view(ml_dtypes.bfloat16).reshape(a.shape)


def _pack_core_inputs(x, wg_e, wu_e, wd_e, toks, ws, nb1=0):
    """Build one core's input map (all matmul operands bf16):
      xT0: [KT/4, 128, 2048]  4 transposed-x k-tiles per DMA
      wgu: [NMC, KT/2, 128, 1024]  [gate 256|up 256] x 2 k-tiles per DMA
      wd:  [NH, KI/2, 128, 1024]  2 k-tiles of a 512-col H chunk per DMA
      rw:  [128, NT] f32 routing weights (token-tile major)"""
    import ml_dtypes
    assert nb1 == 0
    n_e = len(toks)
    xg = np.zeros((B0, H), np.float32)
    xg[:n_e] = x[toks]
    # xT[k] = [128, B0] k-tile; pack groups of 4 along the free dim
    xT = _to_bf16(np.ascontiguousarray(xg.T)).reshape(KT // 4, 4, 128, B0)
    xp = np.ascontiguousarray(xT.transpose(0, 2, 1, 3)).reshape(KT // 4, 128, 4 * B0)
    rfull = np.zeros(B0, np.float32)
    rfull[:n_e] = ws
    # [KT, 128, NMC, 256] per-chunk slices of gate/up
    wg16 = _to_bf16(wg_e).reshape(KT, 128, NMC, 256)
    wu16 = _to_bf16(wu_e).reshape(KT, 128, NMC, 256)
    wgu = np.empty((NMC, KT // 2, 128, 2, 2, 256), ml_dtypes.bfloat16)
    # [mc, kk, p, half(k parity), gate|up, 256]
    wgu[:, :, :, 0, 0] = wg16[0::2].transpose(2, 0, 1, 3)
    wgu[:, :, :, 0, 1] = wu16[0::2].transpose(2, 0, 1, 3)
    wgu[:, :, :, 1, 0] = wg16[1::2].transpose(2, 0, 1, 3)
    wgu[:, :, :, 1, 1] = wu16[1::2].transpose(2, 0, 1, 3)
    # wd: [KI, 128, NH, 512] -> [nh, kk, p, half, 512]
    wd16 = _to_bf16(wd_e).reshape(KI, 128, NH, 512)
    wdp = np.empty((NH, KI // 2, 128, 2, 512), ml_dtypes.bfloat16)
    wdp[:, :, :, 0] = wd16[0::2].transpose(2, 0, 1, 3)
    wdp[:, :, :, 1] = wd16[1::2].transpose(2, 0, 1, 3)
    return {
        "xT0": xp,
        "wgu": wgu.reshape(NMC, KT // 2, 128, 1024),
        "wd": wdp.reshape(NH, KI // 2, 128, 1024),
        "rw": np.ascontiguousarray(rfull.reshape(NT, 128).T),
    }


def kernel(x, w_router, w_gate, w_up, w_down):
    x = np.ascontiguousarray(np.asarray(x, dtype=np.float32))
    w_router = np.asarray(w_router, dtype=np.float32)
    w_gate = np.ascontiguousarray(np.asarray(w_gate, dtype=np.float32))
    w_up = np.ascontiguousarray(np.asarray(w_up, dtype=np.float32))
    w_down = np.ascontiguousarray(np.asarray(w_down, dtype=np.float32))

    if (x.shape != (T, H) or w_router.shape != (H, E)
            or w_gate.shape != (E, H, II) or w_up.shape != (E, H, II)
            or w_down.shape != (E, II, H)):
        return _reference_numpy(x, w_router, w_gate, w_up, w_down)

    top2, w = _route(x, w_router)
    tok = np.repeat(np.arange(T), TOPK)
    te = top2.ravel()
    tw = w.ravel()
    toks_e, ws_e = [], []
    for e in range(E):
        sel = te == e
        toks_e.append(tok[sel])
        ws_e.append(tw[sel].astype(np.float32))

    # Capacity-factor dispatch: the device program handles up to B0=512
    # tokens per expert (the balanced-routing average); the rare spill
    # beyond capacity goes through an exact fp32 host path.
    nc, run_fn = _get_exec(0)

    in_maps = [
        _pack_core_inputs(x, w_gate[e], w_up[e], w_down[e],
                          toks_e[e][:B0], ws_e[e][:B0], 0)
        for e in range(E)
    ]

    try:
        results = run_fn(in_maps)
    except Exception:
        import time as _time
        _time.sleep(20)
        results = run_fn(in_maps)

    out = np.zeros((T, H), np.float32)
    for e in range(E):
        n_e = min(len(toks_e[e]), B0)
        # d: [NH, NT, 128, 512] -> [B0 tokens, H]
        d = results[e]["d"].astype(np.float32).transpose(1, 2, 0, 3).reshape(B0, H)
        out[toks_e[e][:B0]] += d[:n_e]
        spill = toks_e[e][B0:]
        if spill.size:
            xe = x[spill]
            g = xe @ w_gate[e]
            u = xe @ w_up[e]
            a = (g / (1.0 + np.exp(-g))) * u
            out[spill] += (a @ w_down[e]) * ws_e[e][B0:, None]
    return out


# revision 20
# speedup vs baseline: 1.0386x; 1.0386x over previous
"""Trainium2 Bass kernel for nn_BlockSparseMLP (MoE gated MLP, E=8, top-2).

Strategy: expert parallelism over 8 NeuronCores. The router matmul
(x @ w_router, 67 MFLOP out of the 206 GFLOP dense total) plus the
top-2 dispatch/gather and the final scatter-add combine run on the
host; each core runs the full gated MLP (gate/up, silu*up, down,
weighted by the routing prob) for the 512 tokens routed to its expert
(capacity-factor dispatch; overflow beyond 512 tokens/expert takes an
exact fp32 host path). Matmul operands are bf16 (1 col/cycle on the
PE, same rate as float32r, but fast-weight-load keeps LDWEIGHTS off
the critical path and HBM traffic halves); PSUM accumulation and the
silu/mul/scale epilogue run fp32, outputs are stored bf16.

Per-core device program (C = 512 tokens, token-major tiles of 128):
  phase 1 (gate/up): stream w_gate/w_up in [128, gate 256|up 256]
    chunks, two k-tiles per 256KB DMA; weights stationary, xT moving
    (N=512). silu(gate)*up fused on ACT+DVE into aT ([I, C] bf16,
    SBUF-resident, one tile per chunk so phase 2's early reads don't
    depend on the last chunk's writes).
  phase 2 (down): stream w_down in two-k-tile chunks (moving, N=512
    per matmul), aT tiles stationary, accumulate over I into
    [tokens, 512] psum tiles, scale by the per-token routing weight,
    DMA out. The last H-chunk's w_down is fully SBUF-resident (loaded
    mid-phase-2) so that chunk runs token-major and its outputs drain
    one at a time instead of all four serializing after the final
    matmul.

Scheduling notes (each worth microseconds on a 352us kernel):
  - Sync issues ALL streamed weight DMAs and nothing else: weight
    dma_starts block on tile-pool slots once the prefetch races ahead
    of the PE, and anything queued behind them on that engine stalls.
    An earlier revision alternated weight loads sync/scalar; the silu
    for chunk N sat behind chunk N+2's slot-blocked load, psum filled,
    and the PE idled 6us and re-throttled its HAM clock.
  - Scalar: x loads (16 x 128KB at the head; the first matmul only
    waits on one k-tile), then the nh=0 w_down head into a dedicated
    pool (issued after chunk 1's silu, clear of the cold-start HBM
    ramp), then the silus, then most output stores.
  - One PSUM pool spans both phases and the down-projection tiles
    reuse the gate/up tags: FIFO slot recycling then hands phase 2
    banks that were freed a chunk earlier, so the first down matmul
    does not wait for the last chunk's psum drain.
  - GpSimd only loads the tiny routing-weight tile.

Measured on trn2: 352us HW exec (PE stream floor for this shape is
1536 N=512 matmuls = 331.5us; remainder is NEFF preamble ~7us, cold
HAM ramp ~2us, startup DMA ~3us, output tail + drain ~5us, ramp
transients ~3us). Rel err vs the fp32 reference ~4.4e-3.
"""

import sys
import functools

sys.path.insert(0, "/opt/trn_rl_repo")

import numpy as np

T, H, II, E, TOPK = 2048, 2048, 4096, 8, 2
NCORES = 8
B0 = 512        # token capacity per expert (moving N for gate/up)
KT = H // 128   # 16 contraction tiles for gate/up
MTI = II // 128  # 32 I tiles
NMC = II // 256  # 16 weight chunks of [gate 256 | up 256]
KI = II // 128  # 32 contraction tiles for down
NH = H // 512   # 4 output column chunks
NT = B0 // 128  # 4 token tiles


@functools.lru_cache(maxsize=2)
def _build(nb1: int = 0):
    """Build the SPMD Bass program (capacity fixed at B0=512 tokens)."""
    assert nb1 == 0
    import concourse.mybir as mybir
    import concourse.tile as tile
    from concourse import bacc

    f32 = mybir.dt.float32
    bf16 = mybir.dt.bfloat16

    nc = bacc.Bacc(None)
    # x, token-major transposed, packed 4 k-tiles per DMA: [KT/4, 128, 4*B0]
    xT = nc.declare_dram_parameter("xT0", [KT // 4, 128, 4 * B0], bf16, isOutput=False)
    # gate|up chunks, 2 k-tiles per DMA: [NMC, KT/2, 128, 1024]
    wgu = nc.declare_dram_parameter("wgu", [NMC, KT // 2, 128, 1024], bf16, isOutput=False)
    # down chunks, 2 k-tiles per DMA: [NH, KI/2, 128, 1024]
    wd = nc.declare_dram_parameter("wd", [NH, KI // 2, 128, 1024], bf16, isOutput=False)
    rw = nc.declare_dram_parameter("rw", [128, NT], f32, isOutput=False)
    # output, contiguous per (nh, mt) tile: [NH, NT, 128, 512]
    dout = nc.declare_dram_parameter("d", [NH, NT, 128, 512], bf16, isOutput=True)

    SILU = mybir.ActivationFunctionType.Silu

    NPF = 8  # wd pairs of nh=0 prefetched at program start (via GpSimd)

    with tile.TileContext(nc) as tc:
        with (
            tc.tile_pool(name="pers", bufs=1) as pers,
            tc.tile_pool(name="wpool", bufs=14) as wpool,
            tc.tile_pool(name="wd0p", bufs=1) as wd0p,
            tc.tile_pool(name="ps", bufs=1, space="PSUM") as ps,
        ):
            aT0 = [pers.tile([128, 2, B0], bf16, name=f"aT0_{mc}")
                   for mc in range(NMC)]
            rwt = pers.tile([128, NT], f32)
            nc.gpsimd.dma_start(rwt[:], rw[:])
            # nh=0's w_down head gets a dedicated pool: the shared wpool's
            # slots only free near the end of phase 1 (they pace the wgu
            # prefetch), so streaming the first wd chunk through it would
            # stall the phase-1 -> phase-2 handoff. Loads are issued on
            # Scalar after the x loads: early enough to land mid-phase-1,
            # late enough not to steal HBM bandwidth from the startup ramp.
            wd0 = [wd0p.tile([128, 1024], bf16, name=f"wd0_{kk}")
                   for kk in range(NPF)]

            with (
                tc.tile_pool(name="xp", bufs=1) as xp,
                tc.tile_pool(name="sp", bufs=2) as sp,
            ):
                # x as 16 single k-tiles (not wider packs): the first matmul
                # only waits on a 128KB load, and the issues sit on Scalar,
                # which has nothing else to do until the first silu.
                xt = [xp.tile([128, B0], bf16, name=f"xt_{k}") for k in range(KT)]
                for k in range(KT):
                    nc.scalar.dma_start(xt[k][:], xT[k // 4][:, (k % 4) * B0:(k % 4 + 1) * B0])

                for mc in range(NMC):
                    # wb[k] -> (tile, column offset of k's gate|up block)
                    wb = []
                    for kk in range(KT // 2):
                        if mc == 0 and kk == 0:
                            # separate 128KB single-k tiles so the first
                            # matmul waits on one k-tile, not the pair
                            for half in range(2):
                                wbk = wpool.tile([128, 512], bf16,
                                                 name=f"wb0_{half}", bufs=1)
                                nc.sync.dma_start(
                                    wbk[:], wgu[0, 0][:, half * 512:(half + 1) * 512])
                                wb.append((wbk, 0))
                        else:
                            wbk = wpool.tile([128, 1024], bf16, name="wbk", tag="w")
                            nc.sync.dma_start(wbk[:], wgu[mc, kk])
                            wb.append((wbk, 0))
                            wb.append((wbk, 512))

                    pg0 = [ps.tile([128, B0], f32, name="pg0", tag="pg0", bufs=4)
                           for _ in range(2)]
                    pu0 = [ps.tile([128, B0], f32, name="pu0", tag="pu0", bufs=4)
                           for _ in range(2)]

                    for k in range(KT):
                        st = dict(start=(k == 0), stop=(k == KT - 1))
                        wsl, wo = wb[k]
                        for j in range(2):
                            nc.tensor.matmul(
                                pg0[j][:], wsl[:, wo + j * 128:wo + (j + 1) * 128],
                                xt[k][:], **st
                            )
                        for j in range(2):
                            nc.tensor.matmul(
                                pu0[j][:],
                                wsl[:, wo + 256 + j * 128:wo + 256 + (j + 1) * 128],
                                xt[k][:], **st
                            )

                    for j in range(2):
                        sg = sp.tile([128, B0], f32, name="sg", tag="sg")
                        nc.scalar.activation(sg[:], pg0[j][:], SILU)
                        nc.vector.tensor_mul(aT0[mc][:, j, :], sg[:], pu0[j][:])
                    if mc == 1:
                        # nh=0's wd head loads go here on Scalar: after the
                        # startup ramp's bandwidth crunch, long before use
                        for kk in range(NPF):
                            nc.scalar.dma_start(wd0[kk][:], wd[0, kk])

            with (
                tc.tile_pool(name="op", bufs=6) as op,
                tc.tile_pool(name="wlast", bufs=1) as wlast,
            ):
                # The last nh chunk's w_down (4MB) is preloaded whole, so
                # the chunk can run token-major and its outputs drain one
                # at a time instead of all four serializing after the final
                # matmul. Issues go on Sync AFTER nh=0's loads (below), so
                # they execute ~80us before use without competing with the
                # startup ramp.
                wl = [wlast.tile([128, 1024], bf16, name=f"wl_{kk}")
                      for kk in range(KI // 2)]

                for nh in range(NH - 1):
                    pd = [ps.tile([128, 512], f32, name="pd",
                                  tag=("pg0" if mt % 2 == 0 else "pu0"), bufs=4)
                          for mt in range(NT)]
                    for kk in range(KI // 2):
                        if nh == 0 and kk < NPF:
                            wdk = wd0[kk]
                        else:
                            wdk = wpool.tile([128, 1024], bf16, name="wdk", tag="w")
                            nc.sync.dma_start(wdk[:], wd[nh, kk])
                        for half in range(2):
                            k = 2 * kk + half
                            st = dict(start=(k == 0), stop=(k == KI - 1))
                            mv = wdk[:, half * 512:(half + 1) * 512]
                            for mt in range(NT):
                                nc.tensor.matmul(
                                    pd[mt][:],
                                    aT0[k // 2][:, k % 2, mt * 128:(mt + 1) * 128],
                                    mv, **st
                                )
                    if nh == 0:
                        for kk in range(KI // 2):
                            nc.sync.dma_start(wl[kk][:], wd[NH - 1, kk])
                    for mt in range(NT):
                        ot = op.tile([128, 512], bf16, name="ot", tag="ot")
                        nc.vector.tensor_scalar_mul(ot[:], pd[mt][:], rwt[:, mt:mt + 1])
                        nc.scalar.dma_start(dout[nh, mt], ot[:])

                # last nh chunk: token-major, alternating scale/store engines
                nh = NH - 1
                for mt in range(NT):
                    pdt = ps.tile([128, 512], f32, name="pd",
                                  tag=("pg0" if mt % 2 == 0 else "pu0"), bufs=4)
                    for k in range(KI):
                        st = dict(start=(k == 0), stop=(k == KI - 1))
                        nc.tensor.matmul(
                            pdt[:],
                            aT0[k // 2][:, k % 2, mt * 128:(mt + 1) * 128],
                            wl[k // 2][:, (k % 2) * 512:(k % 2 + 1) * 512], **st
                        )
                    ot = op.tile([128, 512], bf16, name="ot", tag="ot")
                    if mt % 2 == 0:
                        nc.vector.tensor_scalar_mul(ot[:], pdt[:], rwt[:, mt:mt + 1])
                        nc.scalar.dma_start(dout[nh, mt], ot[:])
                    else:
                        nc.scalar.activation(
                            ot[:], pdt[:], mybir.ActivationFunctionType.Copy,
                            scale=rwt[:, mt:mt + 1]
                        )
                        nc.sync.dma_start(dout[nh, mt], ot[:])

    nc.compile()
    return nc


@functools.lru_cache(maxsize=2)
def _get_exec(nb1: int = 0):
    """Compile the Bass program and return (nc, run_fn) with a cached jit.

    run_fn(in_maps) -> list of per-core {"d": np.ndarray}. Mirrors
    bass2jax.run_bass_via_pjrt's multi-core branch, but keeps the jitted
    function alive across kernel() calls so repeat invocations skip XLA
    and NEFF compilation.
    """
    import jax
    import concourse.mybir as mybir
    from concourse import bass2jax

    nc = _build(nb1)
    bass2jax.install_neuronx_cc_hook()

    partition_name = nc.partition_id_tensor.name if nc.partition_id_tensor else None
    in_names, out_names, out_avals = [], [], []
    zero_out_shapes = []
    for alloc in nc.m.functions[0].allocations:
        if not isinstance(alloc, mybir.MemoryLocationSet):
            continue
        name = alloc.memorylocations[0].name
        if alloc.kind == "ExternalInput":
            if name != partition_name:
                in_names.append(name)
        elif alloc.kind == "ExternalOutput":
            shape = tuple(alloc.tensor_shape)
            dtype = mybir.dt.np(alloc.dtype)
            out_names.append(name)
            out_avals.append(jax.core.ShapedArray(shape, dtype))
            zero_out_shapes.append((shape, dtype))
    n_params = len(in_names)
    n_outs = len(out_names)
    all_names = list(in_names) + list(out_names)
    if partition_name is not None:
        all_names.append(partition_name)
    donate = tuple(range(n_params, n_params + n_outs))

    def _body(*args):
        operands = list(args)
        if partition_name is not None:
            operands.append(bass2jax.partition_id_tensor())
        outs = bass2jax._bass_exec_p.bind(
            *operands,
            out_avals=tuple(out_avals),
            in_names=tuple(all_names),
            out_names=tuple(out_names),
            lowering_input_output_aliases=(),
            sim_require_finite=True,
            sim_require_nnan=True,
            nc=nc,
        )
        return tuple(outs)

    devices = jax.devices()[:NCORES]
    assert len(devices) == NCORES, f"need {NCORES} devices, have {len(jax.devices())}"
    mesh = bass2jax.Mesh(np.asarray(devices), ("core",))
    in_specs = (bass2jax.PartitionSpec("core"),) * (n_params + n_outs)
    out_specs = (bass2jax.PartitionSpec("core"),) * n_outs
    sharded = jax.jit(
        bass2jax.shard_map(
            _body, mesh=mesh, in_specs=in_specs, out_specs=out_specs, check_rep=False
        ),
        donate_argnums=donate,
        keep_unused=True,
    )

    def run_fn(in_maps):
        concat_in = [
            np.concatenate([np.asarray(m[name]) for m in in_maps], axis=0)
            for name in in_names
        ]
        zeros = [
            np.zeros((shape[0] * NCORES,) + shape[1:], dtype)
            for shape, dtype in zero_out_shapes
        ]
        out_arrs = sharded(*concat_in, *zeros)
        results = []
        for c in range(NCORES):
            res = {}
            for i, name in enumerate(out_names):
                arr = np.asarray(out_arrs[i])
                per = arr.shape[0] // NCORES
                res[name] = arr[c * per:(c + 1) * per]
            results.append(res)
        return results

    return nc, run_fn


def _route(x, w_router):
    """Top-2 routing: expert ids + softmax weights, matching jax.lax.top_k
    (descending, ties to the lower index) + jax.nn.softmax."""
    logits = x.astype(np.float64) @ w_router.astype(np.float64)
    top2 = np.argsort(-logits, axis=1, kind="stable")[:, :TOPK]
    vals = np.take_along_axis(logits, top2, 1).astype(np.float32)
    e = np.exp(vals - vals.max(axis=1, keepdims=True))
    w = (e / e.sum(axis=1, keepdims=True)).astype(np.float32)
    return top2, w


def _reference_numpy(x, w_router, w_gate, w_up, w_down):
    """Correct-but-slow dense fallback for shapes the device program doesn't cover."""
    x = x.astype(np.float32)
    logits = x @ w_router.astype(np.float32)
    n_exp = w_gate.shape[0]
    k = min(TOPK, n_exp)
    top = np.argsort(-logits, axis=1, kind="stable")[:, :k]
    vals = np.take_along_axis(logits, top, 1)
    ex = np.exp(vals - vals.max(1, keepdims=True))
    ww = (ex / ex.sum(1, keepdims=True)).astype(np.float32)
    w_dense = np.zeros_like(logits)
    t_ids = np.arange(x.shape[0])[:, None]
    w_dense[t_ids, top] = ww
    out = np.zeros((x.shape[0], w_down.shape[-1]), np.float32)
    for e in range(n_exp):
        g = x @ w_gate[e]
        u = x @ w_up[e]
        a = (g / (1.0 + np.exp(-g))) * u
        out += w_dense[:, e:e + 1] * (a @ w_down[e])
    return out


def _to_bf16(a):
    """f32 -> bf16 with round-to-nearest-even (vectorized integer trick)."""
    import ml_dtypes
    b = np.ascontiguousarray(a, np.float32).view(np.uint32)
    r = ((b >> 16) & 1) + np.uint32(0x7FFF)
    return ((b + r) >> 16).astype(np.uint16).view(ml_dtypes.bfloat16).reshape(a.shape)


def _pack_core_inputs(x, wg_e, wu_e, wd_e, toks, ws, nb1=0):
    """Build one core's input map (all matmul operands bf16):
      xT0: [KT/4, 128, 2048]  4 transposed-x k-tiles per DMA
      wgu: [NMC, KT/2, 128, 1024]  [gate 256|up 256] x 2 k-tiles per DMA
      wd:  [NH, KI/2, 128, 1024]  2 k-tiles of a 512-col H chunk per DMA
      rw:  [128, NT] f32 routing weights (token-tile major)"""
    import ml_dtypes
    assert nb1 == 0
    n_e = len(toks)
    xg = np.zeros((B0, H), np.float32)
    xg[:n_e] = x[toks]
    # xT[k] = [128, B0] k-tile; pack groups of 4 along the free dim
    xT = _to_bf16(np.ascontiguousarray(xg.T)).reshape(KT // 4, 4, 128, B0)
    xp = np.ascontiguousarray(xT.transpose(0, 2, 1, 3)).reshape(KT // 4, 128, 4 * B0)
    rfull = np.zeros(B0, np.float32)
    rfull[:n_e] = ws
    # [KT, 128, NMC, 256] per-chunk slices of gate/up
    wg16 = _to_bf16(wg_e).reshape(KT, 128, NMC, 256)
    wu16 = _to_bf16(wu_e).reshape(KT, 128, NMC, 256)
    wgu = np.empty((NMC, KT // 2, 128, 2, 2, 256), ml_dtypes.bfloat16)
    # [mc, kk, p, half(k parity), gate|up, 256]
    wgu[:, :, :, 0, 0] = wg16[0::2].transpose(2, 0, 1, 3)
    wgu[:, :, :, 0, 1] = wu16[0::2].transpose(2, 0, 1, 3)
    wgu[:, :, :, 1, 0] = wg16[1::2].transpose(2, 0, 1, 3)
    wgu[:, :, :, 1, 1] = wu16[1::2].transpose(2, 0, 1, 3)
    # wd: [KI, 128, NH, 512] -> [nh, kk, p, half, 512]
    wd16 = _to_bf16(wd_e).reshape(KI, 128, NH, 512)
    wdp = np.empty((NH, KI // 2, 128, 2, 512), ml_dtypes.bfloat16)
    wdp[:, :, :, 0] = wd16[0::2].transpose(2, 0, 1, 3)
    wdp[:, :, :, 1] = wd16[1::2].transpose(2, 0, 1, 3)
    return {
        "xT0": xp,
        "wgu": wgu.reshape(NMC, KT // 2, 128, 1024),
        "wd": wdp.reshape(NH, KI // 2, 128, 1024),
        "rw": np.ascontiguousarray(rfull.reshape(NT, 128).T),
    }


def kernel(x, w_router, w_gate, w_up, w_down):
    x = np.ascontiguousarray(np.asarray(x, dtype=np.float32))
    w_router = np.asarray(w_router, dtype=np.float32)
    w_gate = np.ascontiguousarray(np.asarray(w_gate, dtype=np.float32))
    w_up = np.ascontiguousarray(np.asarray(w_up, dtype=np.float32))
    w_down = np.ascontiguousarray(np.asarray(w_down, dtype=np.float32))

    if (x.shape != (T, H) or w_router.shape != (H, E)
            or w_gate.shape != (E, H, II) or w_up.shape != (E, H, II)
            or w_down.shape != (E, II, H)):
        return _reference_numpy(x, w_router, w_gate, w_up, w_down)

    top2, w = _route(x, w_router)
    tok = np.repeat(np.arange(T), TOPK)
    te = top2.ravel()
    tw = w.ravel()
    toks_e, ws_e = [], []
    for e in range(E):
        sel = te == e
        toks_e.append(tok[sel])
        ws_e.append(tw[sel].astype(np.float32))

    # Capacity-factor dispatch: the device program handles up to B0=512
    # tokens per expert (the balanced-routing average); the rare spill
    # beyond capacity goes through an exact fp32 host path.
    nc, run_fn = _get_exec(0)

    in_maps = [
        _pack_core_inputs(x, w_gate[e], w_up[e], w_down[e],
                          toks_e[e][:B0], ws_e[e][:B0], 0)
        for e in range(E)
    ]

    try:
        results = run_fn(in_maps)
    except Exception:
        import time as _time
        _time.sleep(20)
        results = run_fn(in_maps)

    out = np.zeros((T, H), np.float32)
    for e in range(E):
        n_e = min(len(toks_e[e]), B0)
        # d: [NH, NT, 128, 512] -> [B0 tokens, H]
        d = results[e]["d"].astype(np.float32).transpose(1, 2, 0, 3).reshape(B0, H)
        out[toks_e[e][:B0]] += d[:n_e]
        spill = toks_e[e][B0:]
        if spill.size:
            xe = x[spill]
            g = xe @ w_gate[e]
            u = xe @ w_up[e]
            a = (g / (1.0 + np.exp(-g))) * u
            out[spill] += (a @ w_down[e]) * ws_e[e][B0:, None]
    return out


# revision 21
# speedup vs baseline: 1.0405x; 1.0018x over previous
"""Trainium2 Bass kernel for nn_BlockSparseMLP (MoE gated MLP, E=8, top-2).

Strategy: expert parallelism over 8 NeuronCores. The router matmul
(x @ w_router, 67 MFLOP out of the 206 GFLOP dense total) plus the
top-2 dispatch/gather and the final scatter-add combine run on the
host; each core runs the full gated MLP (gate/up, silu*up, down,
weighted by the routing prob) for the 512 tokens routed to its expert
(capacity-factor dispatch; overflow beyond 512 tokens/expert takes an
exact fp32 host path). Matmul operands are bf16 (1 col/cycle on the
PE, same rate as float32r, but fast-weight-load keeps LDWEIGHTS off
the critical path and HBM traffic halves); PSUM accumulation and the
silu/mul/scale epilogue run fp32, outputs are stored bf16.

Per-core device program (C = 512 tokens, token-major tiles of 128):
  phase 1 (gate/up): stream w_gate/w_up in [128, gate 256|up 256]
    chunks, two k-tiles per 256KB DMA; weights stationary, xT moving
    (N=512). silu(gate)*up fused on ACT+DVE into aT ([I, C] bf16,
    SBUF-resident, one tile per chunk so phase 2's early reads don't
    depend on the last chunk's writes).
  phase 2 (down): stream w_down in two-k-tile chunks (moving, N=512
    per matmul), aT tiles stationary, accumulate over I into
    [tokens, 512] psum tiles, scale by the per-token routing weight,
    DMA out. The last H-chunk's w_down is fully SBUF-resident (loaded
    mid-phase-2) so that chunk runs token-major and its outputs drain
    one at a time instead of all four serializing after the final
    matmul.

Scheduling notes (each worth microseconds on a 352us kernel):
  - Sync issues ALL streamed weight DMAs and nothing else: weight
    dma_starts block on tile-pool slots once the prefetch races ahead
    of the PE, and anything queued behind them on that engine stalls.
    An earlier revision alternated weight loads sync/scalar; the silu
    for chunk N sat behind chunk N+2's slot-blocked load, psum filled,
    and the PE idled 6us and re-throttled its HAM clock.
  - Scalar: x loads (16 x 128KB at the head; the first matmul only
    waits on one k-tile), then the nh=0 w_down head into a dedicated
    pool (issued after chunk 1's silu, clear of the cold-start HBM
    ramp), then the silus, then most output stores.
  - One PSUM pool spans both phases and the down-projection tiles
    reuse the gate/up tags: FIFO slot recycling then hands phase 2
    banks that were freed a chunk earlier, so the first down matmul
    does not wait for the last chunk's psum drain.
  - GpSimd only loads the tiny routing-weight tile.

Measured on trn2: 352us HW exec (PE stream floor for this shape is
1536 N=512 matmuls = 331.5us; remainder is NEFF preamble ~7us, cold
HAM ramp ~2us, startup DMA ~3us, output tail + drain ~5us, ramp
transients ~3us). Rel err vs the fp32 reference ~4.4e-3.
"""

import sys
import functools

sys.path.insert(0, "/opt/trn_rl_repo")

import numpy as np

T, H, II, E, TOPK = 2048, 2048, 4096, 8, 2
NCORES = 8
B0 = 512        # token capacity per expert (moving N for gate/up)
KT = H // 128   # 16 contraction tiles for gate/up
MTI = II // 128  # 32 I tiles
NMC = II // 256  # 16 weight chunks of [gate 256 | up 256]
KI = II // 128  # 32 contraction tiles for down
NH = H // 512   # 4 output column chunks
NT = B0 // 128  # 4 token tiles


@functools.lru_cache(maxsize=2)
def _build(nb1: int = 0):
    """Build the SPMD Bass program (capacity fixed at B0=512 tokens)."""
    assert nb1 == 0
    import concourse.mybir as mybir
    import concourse.tile as tile
    from concourse import bacc

    f32 = mybir.dt.float32
    bf16 = mybir.dt.bfloat16

    nc = bacc.Bacc(None)
    # x, token-major transposed, packed 4 k-tiles per DMA: [KT/4, 128, 4*B0]
    xT = nc.declare_dram_parameter("xT0", [KT // 4, 128, 4 * B0], bf16, isOutput=False)
    # gate|up chunks, 2 k-tiles per DMA: [NMC, KT/2, 128, 1024]
    wgu = nc.declare_dram_parameter("wgu", [NMC, KT // 2, 128, 1024], bf16, isOutput=False)
    # down chunks, 2 k-tiles per DMA: [NH, KI/2, 128, 1024]
    wd = nc.declare_dram_parameter("wd", [NH, KI // 2, 128, 1024], bf16, isOutput=False)
    rw = nc.declare_dram_parameter("rw", [128, NT], f32, isOutput=False)
    # output, contiguous per (nh, mt) tile: [NH, NT, 128, 512]
    dout = nc.declare_dram_parameter("d", [NH, NT, 128, 512], bf16, isOutput=True)

    SILU = mybir.ActivationFunctionType.Silu

    NPF = 8  # wd pairs of nh=0 prefetched at program start (via GpSimd)

    with tile.TileContext(nc) as tc:
        with (
            tc.tile_pool(name="pers", bufs=1) as pers,
            tc.tile_pool(name="wpool", bufs=14) as wpool,
            tc.tile_pool(name="wd0p", bufs=1) as wd0p,
            tc.tile_pool(name="ps", bufs=1, space="PSUM") as ps,
        ):
            aT0 = [pers.tile([128, 2, B0], bf16, name=f"aT0_{mc}")
                   for mc in range(NMC)]
            rwt = pers.tile([128, NT], f32)
            nc.gpsimd.dma_start(rwt[:], rw[:])
            # nh=0's w_down head gets a dedicated pool: the shared wpool's
            # slots only free near the end of phase 1 (they pace the wgu
            # prefetch), so streaming the first wd chunk through it would
            # stall the phase-1 -> phase-2 handoff. Loads are issued on
            # Scalar after the x loads: early enough to land mid-phase-1,
            # late enough not to steal HBM bandwidth from the startup ramp.
            wd0 = [wd0p.tile([128, 1024], bf16, name=f"wd0_{kk}")
                   for kk in range(NPF)]

            with (
                tc.tile_pool(name="xp", bufs=1) as xp,
                tc.tile_pool(name="sp", bufs=2) as sp,
            ):
                # x as 16 single k-tiles (not wider packs): the first matmul
                # only waits on a 128KB load, and the issues sit on Scalar,
                # which has nothing else to do until the first silu.
                xt = [xp.tile([128, B0], bf16, name=f"xt_{k}") for k in range(KT)]
                for k in range(KT):
                    nc.scalar.dma_start(xt[k][:], xT[k // 4][:, (k % 4) * B0:(k % 4 + 1) * B0])

                for mc in range(NMC):
                    # wb[k] -> (tile, column offset of k's gate|up block)
                    wb = []
                    for kk in range(KT // 2):
                        if mc == 0 and kk == 0:
                            # k=0 split 32KB head + 96KB rest: the first
                            # matmul (gate j=0) only needs cols 0:128, and
                            # every us earlier also starts the PE's HAM
                            # warmup window sooner. k=1 is a 128KB single.
                            w00a = wpool.tile([128, 128], bf16, name="w00a", bufs=1)
                            nc.sync.dma_start(w00a[:], wgu[0, 0][:, :128])
                            w00b = wpool.tile([128, 384], bf16, name="w00b", bufs=1)
                            nc.sync.dma_start(w00b[:], wgu[0, 0][:, 128:512])
                            wb.append((w00a, w00b))
                            wbk = wpool.tile([128, 512], bf16,
                                             name="wb0_1", bufs=1)
                            nc.sync.dma_start(wbk[:], wgu[0, 0][:, 512:])
                            wb.append((wbk, 0))
                        else:
                            wbk = wpool.tile([128, 1024], bf16, name="wbk", tag="w")
                            nc.sync.dma_start(wbk[:], wgu[mc, kk])
                            wb.append((wbk, 0))
                            wb.append((wbk, 512))

                    pg0 = [ps.tile([128, B0], f32, name="pg0", tag="pg0", bufs=4)
                           for _ in range(2)]
                    pu0 = [ps.tile([128, B0], f32, name="pu0", tag="pu0", bufs=4)
                           for _ in range(2)]

                    for k in range(KT):
                        st = dict(start=(k == 0), stop=(k == KT - 1))
                        if mc == 0 and k == 0:
                            # split-piece layout: head holds gate j=0,
                            # rest holds [gate j1 | up j0 | up j1]
                            pa, pb = wb[0]
                            gate = [pa[:, 0:128], pb[:, 0:128]]
                            up = [pb[:, 128:256], pb[:, 256:384]]
                        else:
                            wsl, wo = wb[k]
                            gate = [wsl[:, wo + j * 128:wo + (j + 1) * 128]
                                    for j in range(2)]
                            up = [wsl[:, wo + 256 + j * 128:wo + 256 + (j + 1) * 128]
                                  for j in range(2)]
                        for j in range(2):
                            nc.tensor.matmul(pg0[j][:], gate[j], xt[k][:], **st)
                        for j in range(2):
                            nc.tensor.matmul(pu0[j][:], up[j], xt[k][:], **st)

                    for j in range(2):
                        sg = sp.tile([128, B0], f32, name="sg", tag="sg")
                        nc.scalar.activation(sg[:], pg0[j][:], SILU)
                        nc.vector.tensor_mul(aT0[mc][:, j, :], sg[:], pu0[j][:])
                    if mc == 1:
                        # nh=0's wd head loads go here on Scalar: after the
                        # startup ramp's bandwidth crunch, long before use
                        for kk in range(NPF):
                            nc.scalar.dma_start(wd0[kk][:], wd[0, kk])

            with (
                tc.tile_pool(name="op", bufs=6) as op,
                tc.tile_pool(name="wlast", bufs=1) as wlast,
            ):
                # The last nh chunk's w_down (4MB) is preloaded whole, so
                # the chunk can run token-major and its outputs drain one
                # at a time instead of all four serializing after the final
                # matmul. Issues go on Sync AFTER nh=0's loads (below), so
                # they execute ~80us before use without competing with the
                # startup ramp.
                wl = [wlast.tile([128, 1024], bf16, name=f"wl_{kk}")
                      for kk in range(KI // 2)]

                for nh in range(NH - 1):
                    pd = [ps.tile([128, 512], f32, name="pd",
                                  tag=("pg0" if mt % 2 == 0 else "pu0"), bufs=4)
                          for mt in range(NT)]
                    for kk in range(KI // 2):
                        if nh == 0 and kk < NPF:
                            wdk = wd0[kk]
                        else:
                            wdk = wpool.tile([128, 1024], bf16, name="wdk", tag="w")
                            nc.sync.dma_start(wdk[:], wd[nh, kk])
                        for half in range(2):
                            k = 2 * kk + half
                            st = dict(start=(k == 0), stop=(k == KI - 1))
                            mv = wdk[:, half * 512:(half + 1) * 512]
                            for mt in range(NT):
                                nc.tensor.matmul(
                                    pd[mt][:],
                                    aT0[k // 2][:, k % 2, mt * 128:(mt + 1) * 128],
                                    mv, **st
                                )
                    if nh == 0:
                        for kk in range(KI // 2):
                            nc.sync.dma_start(wl[kk][:], wd[NH - 1, kk])
                    for mt in range(NT):
                        ot = op.tile([128, 512], bf16, name="ot", tag="ot")
                        nc.vector.tensor_scalar_mul(ot[:], pd[mt][:], rwt[:, mt:mt + 1])
                        nc.scalar.dma_start(dout[nh, mt], ot[:])

                # last nh chunk: token-major, alternating scale/store engines
                nh = NH - 1
                for mt in range(NT):
                    pdt = ps.tile([128, 512], f32, name="pd",
                                  tag=("pg0" if mt % 2 == 0 else "pu0"), bufs=4)
                    for k in range(KI):
                        st = dict(start=(k == 0), stop=(k == KI - 1))
                        nc.tensor.matmul(
                            pdt[:],
                            aT0[k // 2][:, k % 2, mt * 128:(mt + 1) * 128],
                            wl[k // 2][:, (k % 2) * 512:(k % 2 + 1) * 512], **st
                        )
                    ot = op.tile([128, 512], bf16, name="ot", tag="ot")
                    if mt % 2 == 0:
                        nc.vector.tensor_scalar_mul(ot[:], pdt[:], rwt[:, mt:mt + 1])
                        nc.scalar.dma_start(dout[nh, mt], ot[:])
                    else:
                        nc.scalar.activation(
                            ot[:], pdt[:], mybir.ActivationFunctionType.Copy,
                            scale=rwt[:, mt:mt + 1]
                        )
                        nc.sync.dma_start(dout[nh, mt], ot[:])

    nc.compile()
    return nc


@functools.lru_cache(maxsize=2)
def _get_exec(nb1: int = 0):
    """Compile the Bass program and return (nc, run_fn) with a cached jit.

    run_fn(in_maps) -> list of per-core {"d": np.ndarray}. Mirrors
    bass2jax.run_bass_via_pjrt's multi-core branch, but keeps the jitted
    function alive across kernel() calls so repeat invocations skip XLA
    and NEFF compilation.
    """
    import jax
    import concourse.mybir as mybir
    from concourse import bass2jax

    nc = _build(nb1)
    bass2jax.install_neuronx_cc_hook()

    partition_name = nc.partition_id_tensor.name if nc.partition_id_tensor else None
    in_names, out_names, out_avals = [], [], []
    zero_out_shapes = []
    for alloc in nc.m.functions[0].allocations:
        if not isinstance(alloc, mybir.MemoryLocationSet):
            continue
        name = alloc.memorylocations[0].name
        if alloc.kind == "ExternalInput":
            if name != partition_name:
                in_names.append(name)
        elif alloc.kind == "ExternalOutput":
            shape = tuple(alloc.tensor_shape)
            dtype = mybir.dt.np(alloc.dtype)
            out_names.append(name)
            out_avals.append(jax.core.ShapedArray(shape, dtype))
            zero_out_shapes.append((shape, dtype))
    n_params = len(in_names)
    n_outs = len(out_names)
    all_names = list(in_names) + list(out_names)
    if partition_name is not None:
        all_names.append(partition_name)
    donate = tuple(range(n_params, n_params + n_outs))

    def _body(*args):
        operands = list(args)
        if partition_name is not None:
            operands.append(bass2jax.partition_id_tensor())
        outs = bass2jax._bass_exec_p.bind(
            *operands,
            out_avals=tuple(out_avals),
            in_names=tuple(all_names),
            out_names=tuple(out_names),
            lowering_input_output_aliases=(),
            sim_require_finite=True,
            sim_require_nnan=True,
            nc=nc,
        )
        return tuple(outs)

    devices = jax.devices()[:NCORES]
    assert len(devices) == NCORES, f"need {NCORES} devices, have {len(jax.devices())}"
    mesh = bass2jax.Mesh(np.asarray(devices), ("core",))
    in_specs = (bass2jax.PartitionSpec("core"),) * (n_params + n_outs)
    out_specs = (bass2jax.PartitionSpec("core"),) * n_outs
    sharded = jax.jit(
        bass2jax.shard_map(
            _body, mesh=mesh, in_specs=in_specs, out_specs=out_specs, check_rep=False
        ),
        donate_argnums=donate,
        keep_unused=True,
    )

    def run_fn(in_maps):
        concat_in = [
            np.concatenate([np.asarray(m[name]) for m in in_maps], axis=0)
            for name in in_names
        ]
        zeros = [
            np.zeros((shape[0] * NCORES,) + shape[1:], dtype)
            for shape, dtype in zero_out_shapes
        ]
        out_arrs = sharded(*concat_in, *zeros)
        results = []
        for c in range(NCORES):
            res = {}
            for i, name in enumerate(out_names):
                arr = np.asarray(out_arrs[i])
                per = arr.shape[0] // NCORES
                res[name] = arr[c * per:(c + 1) * per]
            results.append(res)
        return results

    return nc, run_fn


def _route(x, w_router):
    """Top-2 routing: expert ids + softmax weights, matching jax.lax.top_k
    (descending, ties to the lower index) + jax.nn.softmax."""
    logits = x.astype(np.float64) @ w_router.astype(np.float64)
    top2 = np.argsort(-logits, axis=1, kind="stable")[:, :TOPK]
    vals = np.take_along_axis(logits, top2, 1).astype(np.float32)
    e = np.exp(vals - vals.max(axis=1, keepdims=True))
    w = (e / e.sum(axis=1, keepdims=True)).astype(np.float32)
    return top2, w


def _reference_numpy(x, w_router, w_gate, w_up, w_down):
    """Correct-but-slow dense fallback for shapes the device program doesn't cover."""
    x = x.astype(np.float32)
    logits = x @ w_router.astype(np.float32)
    n_exp = w_gate.shape[0]
    k = min(TOPK, n_exp)
    top = np.argsort(-logits, axis=1, kind="stable")[:, :k]
    vals = np.take_along_axis(logits, top, 1)
    ex = np.exp(vals - vals.max(1, keepdims=True))
    ww = (ex / ex.sum(1, keepdims=True)).astype(np.float32)
    w_dense = np.zeros_like(logits)
    t_ids = np.arange(x.shape[0])[:, None]
    w_dense[t_ids, top] = ww
    out = np.zeros((x.shape[0], w_down.shape[-1]), np.float32)
    for e in range(n_exp):
        g = x @ w_gate[e]
        u = x @ w_up[e]
        a = (g / (1.0 + np.exp(-g))) * u
        out += w_dense[:, e:e + 1] * (a @ w_down[e])
    return out


def _to_bf16(a):
    """f32 -> bf16 with round-to-nearest-even (vectorized integer trick)."""
    import ml_dtypes
    b = np.ascontiguousarray(a, np.float32).view(np.uint32)
    r = ((b >> 16) & 1) + np.uint32(0x7FFF)
    return ((b + r) >> 16).astype(np.uint16).view(ml_dtypes.bfloat16).reshape(a.shape)


def _pack_core_inputs(x, wg_e, wu_e, wd_e, toks, ws, nb1=0):
    """Build one core's input map (all matmul operands bf16):
      xT0: [KT/4, 128, 2048]  4 transposed-x k-tiles per DMA
      wgu: [NMC, KT/2, 128, 1024]  [gate 256|up 256] x 2 k-tiles per DMA
      wd:  [NH, KI/2, 128, 1024]  2 k-tiles of a 512-col H chunk per DMA
      rw:  [128, NT] f32 routing weights (token-tile major)"""
    import ml_dtypes
    assert nb1 == 0
    n_e = len(toks)
    xg = np.zeros((B0, H), np.float32)
    xg[:n_e] = x[toks]
    # xT[k] = [128, B0] k-tile; pack groups of 4 along the free dim
    xT = _to_bf16(np.ascontiguousarray(xg.T)).reshape(KT // 4, 4, 128, B0)
    xp = np.ascontiguousarray(xT.transpose(0, 2, 1, 3)).reshape(KT // 4, 128, 4 * B0)
    rfull = np.zeros(B0, np.float32)
    rfull[:n_e] = ws
    # [KT, 128, NMC, 256] per-chunk slices of gate/up
    wg16 = _to_bf16(wg_e).reshape(KT, 128, NMC, 256)
    wu16 = _to_bf16(wu_e).reshape(KT, 128, NMC, 256)
    wgu = np.empty((NMC, KT // 2, 128, 2, 2, 256), ml_dtypes.bfloat16)
    # [mc, kk, p, half(k parity), gate|up, 256]
    wgu[:, :, :, 0, 0] = wg16[0::2].transpose(2, 0, 1, 3)
    wgu[:, :, :, 0, 1] = wu16[0::2].transpose(2, 0, 1, 3)
    wgu[:, :, :, 1, 0] = wg16[1::2].transpose(2, 0, 1, 3)
    wgu[:, :, :, 1, 1] = wu16[1::2].transpose(2, 0, 1, 3)
    # wd: [KI, 128, NH, 512] -> [nh, kk, p, half, 512]
    wd16 = _to_bf16(wd_e).reshape(KI, 128, NH, 512)
    wdp = np.empty((NH, KI // 2, 128, 2, 512), ml_dtypes.bfloat16)
    wdp[:, :, :, 0] = wd16[0::2].transpose(2, 0, 1, 3)
    wdp[:, :, :, 1] = wd16[1::2].transpose(2, 0, 1, 3)
    return {
        "xT0": xp,
        "wgu": wgu.reshape(NMC, KT // 2, 128, 1024),
        "wd": wdp.reshape(NH, KI // 2, 128, 1024),
        "rw": np.ascontiguousarray(rfull.reshape(NT, 128).T),
    }


def kernel(x, w_router, w_gate, w_up, w_down):
    x = np.ascontiguousarray(np.asarray(x, dtype=np.float32))
    w_router = np.asarray(w_router, dtype=np.float32)
    w_gate = np.ascontiguousarray(np.asarray(w_gate, dtype=np.float32))
    w_up = np.ascontiguousarray(np.asarray(w_up, dtype=np.float32))
    w_down = np.ascontiguousarray(np.asarray(w_down, dtype=np.float32))

    if (x.shape != (T, H) or w_router.shape != (H, E)
            or w_gate.shape != (E, H, II) or w_up.shape != (E, H, II)
            or w_down.shape != (E, II, H)):
        return _reference_numpy(x, w_router, w_gate, w_up, w_down)

    top2, w = _route(x, w_router)
    tok = np.repeat(np.arange(T), TOPK)
    te = top2.ravel()
    tw = w.ravel()
    toks_e, ws_e = [], []
    for e in range(E):
        sel = te == e
        toks_e.append(tok[sel])
        ws_e.append(tw[sel].astype(np.float32))

    # Capacity-factor dispatch: the device program handles up to B0=512
    # tokens per expert (the balanced-routing average); the rare spill
    # beyond capacity goes through an exact fp32 host path.
    nc, run_fn = _get_exec(0)

    in_maps = [
        _pack_core_inputs(x, w_gate[e], w_up[e], w_down[e],
                          toks_e[e][:B0], ws_e[e][:B0], 0)
        for e in range(E)
    ]

    try:
        results = run_fn(in_maps)
    except Exception:
        import time as _time
        _time.sleep(20)
        results = run_fn(in_maps)

    out = np.zeros((T, H), np.float32)
    for e in range(E):
        n_e = min(len(toks_e[e]), B0)
        # d: [NH, NT, 128, 512] -> [B0 tokens, H]
        d = results[e]["d"].astype(np.float32).transpose(1, 2, 0, 3).reshape(B0, H)
        out[toks_e[e][:B0]] += d[:n_e]
        spill = toks_e[e][B0:]
        if spill.size:
            xe = x[spill]
            g = xe @ w_gate[e]
            u = xe @ w_up[e]
            a = (g / (1.0 + np.exp(-g))) * u
            out[spill] += (a @ w_down[e]) * ws_e[e][B0:, None]
    return out
